# revision 27
# baseline (speedup 1.0000x reference)
"""Trainium2 Bass kernel for nn_DecoderBlock (gnn_message_passing).

Sharding: data-parallel over batch B=16 across 8 cores (2 batches/core).
The knn gather tables (LN(q)@knn_w1, LN(v)@cknn_w1) are computed shard-wise
and assembled with an 8-core AllGather, so gathers use global indices
directly (no rebasing).

Matmuls run as float32r (TF32-like, 1 cycle/row at N>=256 vs 4 for fp32);
every matmul operand is produced by a compute op writing through an
fp32r-bitcast AP so the BIR verifier sees rounded producers.

Attention is computed fully transposed: scores S^T[m,n] per head pair
(row-packed on the PE), exp without max-subtraction (scores are O(+-5)),
attn@V with V stationary (N=512), softmax normalization deferred to a
rank-1 reciprocal broadcast matmul applied at PSUM eviction.

Self-contained: hardcodes all shapes; do not import sibling files.
"""

import numpy as np

import concourse.bass as bass
import concourse.tile as tile
from concourse import mybir
from concourse import bass_utils
from concourse.masks import make_identity
from concourse.tile import ScopedClock

# ---------------- problem dims (hardcoded) ----------------
B, N, NV, D, H, K = 16, 1024, 1024, 384, 6, 8
HD = D // H  # 64
EPS = 1e-5
NCORES = 8
BS = B // NCORES          # 2 local batches per core
NL = BS * N               # 2048 local rows
P = 128
CT = D // P               # 3 c-tiles of 128
NT = N // P               # 8 n-tiles per batch
FT = 4 * D // P           # 12 fc1-out tiles
F32 = mybir.dt.float32
F32R = mybir.dt.float32r
I32 = mybir.dt.int32

USE_FP32R = True

MAX_WAITS_PER_INST = 1


def RR(ap):
    return ap.bitcast(F32R) if USE_FP32R else ap


class _TileContext(tile.TileContext):
    """Kernel-tail drain must not carry >1 sync-wait per instruction on
    this walrus build (setupSyncWait limit); spread the waits over NOPs."""

    def _drain_and_barrier(self, tick_clock, wait_clock):
        carrier = self.nc.sync.nop(nofuse=True)
        wait_clock.add_sem_waits(
            carrier.ins, ScopedClock({None: tick_clock.global_clock})
        )
        waits = list(carrier.ins.sync_info.on_wait)
        if len(waits) > MAX_WAITS_PER_INST:
            carrier.ins.sync_info = mybir.SyncInfo(
                on_wait=waits[:MAX_WAITS_PER_INST], on_update=[]
            )
            for i in range(MAX_WAITS_PER_INST, len(waits), MAX_WAITS_PER_INST):
                extra = self.nc.sync.nop(nofuse=True)
                extra.ins.sync_info = mybir.SyncInfo(
                    on_wait=waits[i : i + MAX_WAITS_PER_INST], on_update=[]
                )
        self.nc.sync.drain()
        self.nc.all_engine_barrier()
        assert self.sems is not None
        popped = self.nc._tile_sem_poison_stack.pop()
        assert popped is self._sem_poison
        self.nc.clear_and_free_semaphores(list(self.sems.allocated().values()))
        self.nc.all_engine_barrier()

    def _lower_ordered_insts(self, ordered):
        # walrus caps sync-wait commands per instruction (~4). Wide HWDGE
        # DMAs fan out across 8 queue sems, so consumers can accumulate 8+
        # waits. Move excess waits onto injected same-engine NOPs.
        for bbname, insts in ordered.items():
            i = 0
            while i < len(insts):
                inst = insts[i]
                si = inst.sync_info
                # walrus allows at most 2 sync commands (waits + updates)
                CAP = max(0, 2 - len(si.on_update)) if si is not None else 2
                if si is not None and len(si.on_wait) > CAP:
                    waits = list(si.on_wait)
                    inst.sync_info = mybir.SyncInfo(
                        on_wait=waits[:CAP], on_update=list(si.on_update))
                    extra = waits[CAP:]
                    pos = i
                    for j in range(0, len(extra), CAP):
                        nop = mybir.InstNoOp(
                            name=self.nc.get_next_instruction_name(),
                            ins=[], outs=[])
                        nop.engine = inst.engine
                        nop.sync_info = mybir.SyncInfo(
                            on_wait=extra[j:j + CAP], on_update=[])
                        self.nc.register_instruction(nop, overwrite=True)
                        insts.insert(pos, nop)
                        pos += 1
                        i += 1
                i += 1
        return super()._lower_ordered_insts(ordered)


def _bcast(t, length, offset=0):
    ap = t.ap()
    return bass.AP(tensor=ap.tensor, offset=ap.offset + offset,
                   ap=[[0, P], [1, length]])


AF = mybir.ActivationFunctionType
OP = mybir.AluOpType


def build_module(mock_collective=False):
    nc = bass.Bass("TRN2", target_bir_lowering=False, debug=False,
                   num_devices=NCORES)
    nc._mock_collective = mock_collective

    q_loc = nc.dram_tensor("q_loc", [NL, D], F32, kind="ExternalInput")
    v_loc = nc.dram_tensor("v_loc", [NL, D], F32, kind="ExternalInput")
    idx1 = nc.dram_tensor("idx1", [P, P], I32, kind="ExternalInput")
    idx2 = nc.dram_tensor("idx2", [P, P], I32, kind="ExternalInput")

    w = {}
    def win(name, shape):
        w[name] = nc.dram_tensor(name, shape, F32, kind="ExternalInput")
    win("n1_g", [D]); win("n1_b", [D])
    win("qkv_w", [D, 3 * D])
    win("sproj_w", [D, D]); win("sproj_b", [D])
    win("nq_g", [D]); win("nq_b", [D])
    win("nv_g", [D]); win("nv_b", [D])
    win("q_w", [D, D]); win("k_w", [D, D]); win("v_w", [D, D])
    win("cproj_w", [D, D]); win("cproj_b", [D])
    win("knn_w", [2 * D, D]); win("knn_b", [D])
    win("merge_w", [2 * D, D]); win("merge_b", [D])
    win("cknn_w", [2 * D, D]); win("cknn_b", [D])
    win("cmerge_w", [2 * D, D]); win("cmerge_b", [D])
    win("n2_g", [D]); win("n2_b", [D])
    win("fc1_w", [D, 4 * D]); win("fc1_b", [4 * D])
    win("fc2_w", [4 * D, D]); win("fc2_b", [D])

    out = nc.dram_tensor("out", [NL, D], F32, kind="ExternalOutput")

    # internal DRAM (dedicated tensors: indirect DMA needs offset-0 APs)
    y1_loc = nc.dram_tensor("y1_loc", [NL, D], F32)
    y1_full = nc.dram_tensor("y1_full", [B * N, D], F32, addr_space="Shared")
    y2_loc = nc.dram_tensor("y2_loc", [NL, D], F32)
    y2_full = nc.dram_tensor("y2_full", [B * NV, D], F32, addr_space="Shared")
    q2_d = nc.dram_tensor("q2_d", [NL, D], F32)   # q after block1
    q3_d = nc.dram_tensor("q3_d", [NL, D], F32)   # q after block2
    bias_bounce = nc.dram_tensor("bias_bounce", [2, D], F32)

    with _TileContext(nc) as tc:
        with nc.allow_low_precision(
                reason="fp32r matmul operands (TF32-like) by design"):
            _build(nc, tc, locals())
    return nc


def _build(nc, tc, T):
    w = T["w"]
    q_loc, v_loc = T["q_loc"], T["v_loc"]
    idx1, idx2 = T["idx1"], T["idx2"]
    out = T["out"]
    y1_loc, y1_full = T["y1_loc"], T["y1_full"]
    y2_loc, y2_full = T["y2_loc"], T["y2_full"]
    q2_d, q3_d = T["q2_d"], T["q3_d"]
    bias_bounce = T["bias_bounce"]

    import contextlib
    with contextlib.ExitStack() as ctx:
        const = ctx.enter_context(tc.tile_pool(name="const", bufs=1))
        work = ctx.enter_context(tc.tile_pool(name="work", bufs=2))
        gpool = ctx.enter_context(tc.tile_pool(name="gpool", bufs=1))
        small = ctx.enter_context(tc.tile_pool(name="small", bufs=4))
        tpool = ctx.enter_context(tc.tile_pool(name="tpool", bufs=2))
        # PSUM budget (8 banks): sps 2 + po 2 + mmB 2 + tp 2
        psS = ctx.enter_context(tc.tile_pool(name="psS", bufs=1, space="PSUM"))
        psO = ctx.enter_context(tc.tile_pool(name="psO", bufs=2, space="PSUM"))
        psB = ctx.enter_context(tc.tile_pool(name="psB", bufs=2, space="PSUM"))
        psT = ctx.enter_context(tc.tile_pool(name="psT", bufs=2, space="PSUM"))

        ident = const.tile([P, P], F32)
        make_identity(nc, ident[:])
        eps_t = const.tile([P, 1], F32)
        nc.vector.memset(eps_t[:], EPS)
        ones_raw = const.tile([P, HD], F32)
        nc.vector.memset(ones_raw[:], 1.0)
        ones_t = const.tile([P, HD], F32)
        nc.vector.tensor_copy(out=RR(ones_t[:]), in_=ones_raw[:])
        onesva_raw = const.tile([P, NT, H, 1], F32)
        nc.vector.memset(onesva_raw[:], 1.0)

        def ln_cols(gname, bname):
            t = const.tile([P, 2, CT], F32, name=f"lncol_{gname}")
            for c in range(CT):
                nc.sync.dma_start(out=t[:, 0, c:c + 1],
                                  in_=w[gname].ap()[c * P:(c + 1) * P, None])
                nc.sync.dma_start(out=t[:, 1, c:c + 1],
                                  in_=w[bname].ap()[c * P:(c + 1) * P, None])
            return t

        n1 = ln_cols("n1_g", "n1_b")
        nqc = ln_cols("nq_g", "nq_b")
        nvc = ln_cols("nv_g", "nv_b")
        n2c = ln_cols("n2_g", "n2_b")

        def brow(name, length=D):
            t = const.tile([P, length], F32, name=f"brow_{name}")
            nc.sync.dma_start(out=t[:], in_=_bcast(w[name], length))
            return t

        knn_b_t = brow("knn_b")
        cknn_b_t = brow("cknn_b")
        fc2_b_t = brow("fc2_b")

        fc1b = const.tile([P, FT], F32)
        for c in range(FT):
            nc.sync.dma_start(out=fc1b[:, c:c + 1],
                              in_=w["fc1_b"].ap()[c * P:(c + 1) * P, None])

        idx1_sb = const.tile([P, P], I32, name="idx1_sb")
        nc.sync.dma_start(out=idx1_sb[:], in_=idx1.ap())
        idx2_sb = const.tile([P, P], I32, name="idx2_sb")
        nc.sync.dma_start(out=idx2_sb[:], in_=idx2.ap())

        bcomb = const.tile([P, 2, D], F32, name="bcomb")

        # ---------------- shared helpers ----------------
        def wtiles(pool, name, rows, cols, row_off=0):
            """Load weight rows as [128, rows/128, cols], rounded to fp32r.
            The BIR verifier needs the matmul operand's producer to be a
            rounding compute op, so DMA into a staging tile and round-copy
            into the real one (in-place copies don't satisfy it)."""
            t = pool.tile([P, rows // P, cols], F32,
                          name=f"w_{name}_{row_off}", tag=f"w_{name}_{row_off}")
            if not USE_FP32R:
                nc.sync.dma_start(
                    out=t[:],
                    in_=w[name].ap()[row_off:row_off + rows, :].rearrange(
                        "(t p) c -> p t c", p=P))
                return t
            for rt in range(rows // P):
                stage = pool.tile([P, cols], F32, name="wstage", tag="wstage",
                                  bufs=1)
                nc.sync.dma_start(
                    out=stage[:],
                    in_=w[name].ap()[row_off + rt * P:row_off + (rt + 1) * P, :])
                nc.vector.tensor_copy(out=RR(t[:, rt, :]), in_=stage[:])
            return t

        def mm_acc(psum, lhsT_list, rhs_list):
            n = len(lhsT_list)
            for i, (lt, rh) in enumerate(zip(lhsT_list, rhs_list)):
                nc.tensor.matmul(psum, lhsT=RR(lt), rhs=RR(rh), start=(i == 0),
                                 stop=(i == n - 1))

        def combine(pool, dst_Wc_half, proj_name, m1_sb, proj_b_name,
                    merge_b_name, slot):
            """dst_Wc_half[:, h, :] ([64, 2H, D]) = (proj_w @ M1) rows for
            head h; bcomb[slot] = proj_b@M1 + merge_b."""
            proj_sb = wtiles(pool, proj_name, D, D)
            projT = pool.tile([P, CT, D], F32, name=f"projT{slot}")
            for cc in range(CT):
                for jj in range(CT):
                    pt = psT.tile([P, P], F32, name="tp", tag="tp")
                    nc.tensor.transpose(
                        out=pt[:], in_=proj_sb[:, cc, jj * P:(jj + 1) * P],
                        identity=ident[:])
                    nc.vector.tensor_copy(
                        out=RR(projT[:, jj, cc * P:(cc + 1) * P]), in_=pt[:])
            for h in range(H):
                pm = psB.tile([HD, D], F32, name="wch_ps", tag="mmB")
                mm_acc(pm[:],
                       [projT[:, jj, h * HD:(h + 1) * HD] for jj in range(CT)],
                       [m1_sb[:, jj, :] for jj in range(CT)])
                nc.vector.tensor_copy(out=RR(dst_Wc_half[:, h, :]), in_=pm[:])
            pb0 = pool.tile([P, CT], F32, name=f"pbcol0{slot}")
            for c in range(CT):
                nc.sync.dma_start(
                    out=pb0[:, c:c + 1],
                    in_=w[proj_b_name].ap()[c * P:(c + 1) * P, None])
            pb = pool.tile([P, CT], F32, name=f"pbcol{slot}")
            nc.vector.tensor_copy(out=RR(pb[:]), in_=pb0[:])
            pmb = psB.tile([1, D], F32, name="mmBb", tag="mmB")
            mm_acc(pmb[:], [pb[:, jj:jj + 1] for jj in range(CT)],
                   [m1_sb[:, jj, :] for jj in range(CT)])
            mb = pool.tile([1, D], F32, name=f"mb{slot}")
            nc.sync.dma_start(out=mb[:], in_=w[merge_b_name].ap()[None, :])
            bc = pool.tile([1, D], F32, name=f"bc{slot}")
            nc.vector.tensor_tensor(out=bc[:], in0=pmb[:], in1=mb[:], op=OP.add)
            nc.sync.dma_start(out=bias_bounce.ap()[slot:slot + 1, :], in_=bc[:])
            nc.sync.dma_start(out=bcomb[:, slot, :],
                              in_=_bcast(bias_bounce, D, offset=slot * D))

        def ln_transpose(get_src, lncol, dstT, nt):
            """LN one [128, D] row-tile; write transposed (g,b applied as
            per-partition scalars, rounded fp32r) into dstT[:, ct, nt*128:]."""
            xt = get_src()
            stats = small.tile([P, 6], F32, name="ln_stats", tag="ln_stats")
            mv = small.tile([P, 2], F32, name="ln_mv", tag="ln_mv")
            nc.vector.bn_stats(out=stats[:], in_=xt)
            nc.vector.bn_aggr(out=mv[:], in_=stats[:])
            rstd = small.tile([P, 1], F32, name="ln_rstd", tag="ln_rstd")
            nc.scalar.activation(out=rstd[:], in_=mv[:, 1:2], func=AF.Sqrt,
                                 bias=eps_t[:], scale=1.0)
            nc.vector.reciprocal(out=rstd[:], in_=rstd[:])
            z = work.tile([P, D], F32, name="ln_z", tag="tmpA")
            nc.vector.tensor_scalar(out=z[:], in0=xt, scalar1=mv[:, 0:1],
                                    scalar2=rstd[:], op0=OP.subtract, op1=OP.mult)
            pt3 = psT.tile([P, D], F32, name="lnT", tag="tp")
            for ct in range(CT):
                nc.tensor.transpose(out=pt3[:, ct * P:(ct + 1) * P],
                                    in_=z[:, ct * P:(ct + 1) * P],
                                    identity=ident[:])
            for ct in range(CT):
                nc.vector.tensor_scalar(out=RR(dstT[:, ct, nt * P:(nt + 1) * P]),
                                        in0=pt3[:, ct * P:(ct + 1) * P],
                                        scalar1=lncol[:, 0, ct:ct + 1],
                                        scalar2=lncol[:, 1, ct:ct + 1],
                                        op0=OP.mult, op1=OP.add)

        def table_shard(apool, xT, W1l, y_loc_t, b):
            # stage the whole batch shard in SBUF, store with ONE DMA so the
            # AllGather input carries a single sem wait (walrus sync-wait cap).
            # Shares the V_aug slot (phases are sequential).
            ysb = apool.tile([P, NT, D], F32, name="y_sb", tag="V_aug", bufs=1)
            for nt in range(NT):
                pm = psB.tile([P, D], F32, name="y_ps", tag="mmB")
                mm_acc(pm[:],
                       [xT[:, ct, nt * P:(nt + 1) * P] for ct in range(CT)],
                       [W1l[:, ct, :] for ct in range(CT)])
                nc.vector.tensor_copy(out=ysb[:, nt, :], in_=pm[:])
            nc.sync.dma_start(
                out=y_loc_t.ap()[b * N:(b + 1) * N, :].rearrange(
                    "(t p) c -> p t c", p=P),
                in_=ysb[:])

        def attention(apool, xqT, xvT, wq, wk, wv, O_norm):
            """O_norm [64, H, N]: per-head normalized attention output,
            head h rows on partitions 0..63 (transposed layout)."""
            (wq_t, wq_off), (wk_t, wk_off), (wv_t, wv_off) = wq, wk, wv

            V_aug = apool.tile([P, NT, H * (HD + 1)], F32, name="V_aug",
                               tag="V_aug")
            ones_cols = V_aug[:].rearrange("p m (h c) -> p m h c",
                                           h=H)[:, :, :, HD:HD + 1]
            nc.vector.tensor_copy(out=RR(ones_cols), in_=onesva_raw[:])
            for mt in range(NT):
                pmv = psB.tile([P, D], F32, name="v_ps", tag="mmB")
                mm_acc(pmv[:],
                       [xvT[:, ct, mt * P:(mt + 1) * P] for ct in range(CT)],
                       [wv_t[:, ct, wv_off:wv_off + D] for ct in range(CT)])
                nc.vector.tensor_copy(
                    out=RR(V_aug[:, mt, :].rearrange("p (h c) -> p h c",
                                                     h=H)[:, :, 0:HD]),
                    in_=pmv[:].rearrange("p (h c) -> p h c", h=H))

            for hp in range(H // 2):
                QT = apool.tile([P, N], F32, name="QT", tag="QT")
                KT = apool.tile([P, N], F32, name="KT", tag="KT")
                for ch in range(2):
                    pmq = psB.tile([P, N // 2], F32, name="qt_ps", tag="mmB")
                    mm_acc(pmq[:],
                           [wq_t[:, ct, wq_off + hp * P:wq_off + (hp + 1) * P]
                            for ct in range(CT)],
                           [xqT[:, ct, ch * 512:(ch + 1) * 512]
                            for ct in range(CT)])
                    nc.vector.tensor_copy(out=RR(QT[:, ch * 512:(ch + 1) * 512]),
                                          in_=pmq[:])
                    pmk = psB.tile([P, N // 2], F32, name="kt_ps", tag="mmB")
                    mm_acc(pmk[:],
                           [wk_t[:, ct, wk_off + hp * P:wk_off + (hp + 1) * P]
                            for ct in range(CT)],
                           [xvT[:, ct, ch * 512:(ch + 1) * 512]
                            for ct in range(CT)])
                    nc.vector.tensor_copy(out=RR(KT[:, ch * 512:(ch + 1) * 512]),
                                          in_=pmk[:])

                for ch in range(2):  # n chunks of 512
                    po = [psO.tile([HD + 1, 512], F32, name=f"po{hh}",
                                   tag="po") for hh in range(2)]
                    for mt in range(NT):
                        sps = psS.tile([P, 2, 512], F32, name="sps", tag="sps")
                        for hh in range(2):
                            nc.tensor.matmul(
                                sps[:, hh, :],
                                lhsT=RR(KT[hh * HD:(hh + 1) * HD,
                                           mt * P:(mt + 1) * P]),
                                rhs=RR(QT[hh * HD:(hh + 1) * HD,
                                          ch * 512:(ch + 1) * 512]),
                                start=True, stop=True,
                                tile_position=(hh * HD, 0))
                        e_t = apool.tile([P, 2, 512], F32, name="e_t",
                                         tag="e_t", bufs=1)
                        nc.scalar.activation(out=RR(e_t[:]), in_=sps[:],
                                             func=AF.Exp,
                                             scale=float(HD) ** -0.5)
                        for hh in range(2):
                            h = hp * 2 + hh
                            nc.tensor.matmul(
                                po[hh][:],
                                lhsT=RR(V_aug[:, mt,
                                              h * (HD + 1):(h + 1) * (HD + 1)]),
                                rhs=RR(e_t[:, hh, :]),
                                start=(mt == 0), stop=(mt == NT - 1))
                    for hh in range(2):
                        h = hp * 2 + hh
                        rT = apool.tile([HD + 1, 512], F32, name="rT",
                                        tag="rT", bufs=1)
                        nc.vector.reciprocal(out=RR(rT[HD:HD + 1, :]),
                                             in_=po[hh][HD:HD + 1, :])
                        Rm = psT.tile([P, 512], F32, name="Rm", tag="tp")
                        nc.tensor.matmul(Rm[0:HD, :],
                                         lhsT=RR(ones_t[HD:HD + 1, :]),
                                         rhs=RR(rT[HD:HD + 1, :]),
                                         start=True, stop=True)
                        # only one TensorTensor input may come from PSUM
                        Rs = apool.tile([HD, 512], F32, name="Rs", tag="Rs",
                                        bufs=2)
                        nc.vector.tensor_copy(out=Rs[:], in_=Rm[0:HD, :])
                        nc.vector.tensor_tensor(
                            out=RR(O_norm[:, h, ch * 512:(ch + 1) * 512]),
                            in0=po[hh][0:HD, :], in1=Rs[:], op=OP.mult)

        def knn_max_gather(idx_sb, y_table, b, nt):
            """Gather K neighbor rows to [128, K, D]; strided max tree
            (levels 1-2 on GpSimd, final on DVE)."""
            gb = gpool.tile([P, K, D], F32, name="gb", tag="gb", bufs=2)
            for k in range(K):
                col = b * 64 + k * 8 + nt
                nc.gpsimd.indirect_dma_start(
                    out=gb[:, k, :], out_offset=None, in_=y_table.ap(),
                    in_offset=bass.IndirectOffsetOnAxis(
                        ap=idx_sb[:, col:col + 1], axis=0))
            # pairwise first level: each op waits on only 2 gather DMAs
            # (walrus caps sync-wait commands per instruction)
            m4 = gpool.tile([P, K // 2, D], F32, name="m4", tag="m4")
            for j in range(K // 2):
                nc.vector.tensor_tensor(out=m4[:, j, :], in0=gb[:, 2 * j, :],
                                        in1=gb[:, 2 * j + 1, :], op=OP.max)
            nc.vector.tensor_tensor(out=m4[:, 0, :], in0=m4[:, 0, :],
                                    in1=m4[:, 1, :], op=OP.max)
            nc.vector.tensor_tensor(out=m4[:, 2, :], in0=m4[:, 2, :],
                                    in1=m4[:, 3, :], op=OP.max)
            g1 = gpool.tile([P, D], F32, name="g1", tag="g1")
            nc.vector.tensor_tensor(out=g1[:], in0=m4[:, 0, :],
                                    in1=m4[:, 2, :], op=OP.max)
            return g1

        def block(bi, xq_src_dram, lncol_q, xv_src_dram, lncol_v, wq, wk, wv,
                  W1l, WDl, knnb_t, Wc_half, M2l, bslot,
                  idx_sb, y_loc_t, y_full_t, q_in_dram, q_out_dram):
            with tc.tile_pool(name=f"blk{bi}", bufs=1) as bp:
                xqT = bp.tile([P, BS, CT, N], F32, name="xqT")
                xvT = (xqT if xv_src_dram is None else
                       bp.tile([P, BS, CT, N], F32, name="xvT"))
                for b in range(BS):
                    for nt in range(NT):
                        def load_q(b=b, nt=nt):
                            t = work.tile([P, D], F32, name="x_ld", tag="x_ld")
                            nc.sync.dma_start(
                                out=t[:],
                                in_=xq_src_dram.ap()[
                                    (b * NT + nt) * P:(b * NT + nt + 1) * P, :])
                            return t[:]
                        ln_transpose(load_q, lncol_q, xqT[:, b], nt)
                        if xv_src_dram is not None:
                            def load_v(b=b, nt=nt):
                                t = work.tile([P, D], F32, name="x_ld2",
                                              tag="x_ld")
                                nc.sync.dma_start(
                                    out=t[:],
                                    in_=xv_src_dram.ap()[
                                        (b * NT + nt) * P:
                                        (b * NT + nt + 1) * P, :])
                                return t[:]
                            ln_transpose(load_v, lncol_v, xvT[:, b], nt)
                for b in range(BS):
                    table_shard(bp, xvT[:, b], W1l, y_loc_t, b)
                if getattr(nc, "_mock_collective", False):
                    for cc in range(NCORES):
                        nc.sync.dma_start(
                            out=y_full_t.ap()[cc * NL:(cc + 1) * NL, :],
                            in_=y_loc_t.ap())
                else:
                    nc.gpsimd.collective_compute(
                        "AllGather", OP.bypass,
                        replica_groups=[list(range(NCORES))],
                        ins=[y_loc_t.ap()], outs=[y_full_t.ap()])

                O_norm = bp.tile([HD, H, N], F32, name="O_norm")
                for b in range(BS):
                    attention(bp, xqT[:, b], xvT[:, b], wq, wk, wv, O_norm)
                    for nt in range(NT):
                        g = knn_max_gather(idx_sb, y_full_t, b, nt)
                        pmb = psB.tile([P, D], F32, name="base_ps", tag="mmB")
                        mm_acc(pmb[:],
                               [xqT[:, b, ct, nt * P:(nt + 1) * P]
                                for ct in range(CT)],
                               [WDl[:, ct, :] for ct in range(CT)])
                        t1 = work.tile([P, D], F32, name="knn_t", tag="tmpA")
                        nc.vector.tensor_tensor(out=t1[:], in0=pmb[:], in1=g[:],
                                                op=OP.add)
                        nc.vector.tensor_tensor(out=t1[:], in0=t1[:],
                                                in1=knnb_t[:], op=OP.add)
                        t2 = work.tile([P, D], F32, name="knn_t2", tag="tmpB")
                        nc.scalar.mul(out=t2[:], in_=t1[:], mul=0.2)
                        nc.vector.tensor_tensor(out=t1[:], in0=t1[:], in1=t2[:],
                                                op=OP.max)
                        # transpose kf[nt] (3 tiles into one psum bank)
                        kfT = tpool.tile([P, CT * P], F32, name="kfTt",
                                         tag="kfTt")
                        ptk = psT.tile([P, D], F32, name="kfT_ps", tag="tp")
                        for ct in range(CT):
                            nc.tensor.transpose(
                                out=ptk[:, ct * P:(ct + 1) * P],
                                in_=t1[:, ct * P:(ct + 1) * P],
                                identity=ident[:])
                        nc.vector.tensor_copy(out=RR(kfT[:]), in_=ptk[:])
                        # merge: 6 half-matmuls (O side) + 3 (kf side)
                        pm = psB.tile([P, D], F32, name="mg_ps", tag="mmB")
                        for h in range(H):
                            nc.tensor.matmul(
                                pm[:], lhsT=RR(O_norm[:, h, nt * P:(nt + 1) * P]),
                                rhs=RR(Wc_half[:, h, :]),
                                start=(h == 0), stop=False)
                        for ct in range(CT):
                            nc.tensor.matmul(
                                pm[:], lhsT=RR(kfT[:, ct * P:(ct + 1) * P]),
                                rhs=RR(M2l[:, ct, :]), start=False,
                                stop=(ct == CT - 1))
                        qin = work.tile([P, D], F32, name="qin", tag="x_ld")
                        nc.sync.dma_start(
                            out=qin[:],
                            in_=q_in_dram.ap()[
                                (b * NT + nt) * P:(b * NT + nt + 1) * P, :])
                        t3 = work.tile([P, D], F32, name="mg_t", tag="tmpA")
                        nc.vector.tensor_tensor(out=t3[:], in0=pm[:],
                                                in1=bcomb[:, bslot, :],
                                                op=OP.add)
                        nc.vector.tensor_tensor(out=t3[:], in0=t3[:],
                                                in1=qin[:], op=OP.add)
                        nc.sync.dma_start(
                            out=q_out_dram.ap()[
                                (b * NT + nt) * P:(b * NT + nt + 1) * P, :],
                            in_=t3[:])

        # ================= block 1 (self) =================
        with tc.tile_pool(name="w1pool", bufs=1) as wp:
            qkv_w = wtiles(wp, "qkv_w", D, 3 * D)
            W1 = wtiles(wp, "knn_w", D, D, 0)
            M2 = wtiles(wp, "merge_w", D, D, D)
            WD = wp.tile([P, CT, D], F32, name="WD")
            Wc1 = wp.tile([HD, 2 * H, D], F32, name="Wc1")
            with tc.tile_pool(name="w1tmp", bufs=1) as wt:
                W2 = wtiles(wt, "knn_w", D, D, D)
                for c in range(CT):
                    nc.vector.tensor_tensor(out=RR(WD[:, c, :]), in0=W2[:, c, :],
                                            in1=W1[:, c, :], op=OP.subtract)
                M1 = wtiles(wt, "merge_w", D, D, 0)
                combine(wt, Wc1, "sproj_w", M1, "sproj_b", "merge_b", 0)

            block(1, q_loc, n1, None, None,
                  (qkv_w, 0), (qkv_w, D), (qkv_w, 2 * D),
                  W1, WD, knn_b_t, Wc1, M2, 0,
                  idx1_sb, y1_loc, y1_full, q_loc, q2_d)

        # ================= block 2 (cross) =================
        with tc.tile_pool(name="w2pool", bufs=1) as wp:
            qw_t = wtiles(wp, "q_w", D, D)
            kw_t = wtiles(wp, "k_w", D, D)
            vw_t = wtiles(wp, "v_w", D, D)
            C1 = wtiles(wp, "cknn_w", D, D, 0)
            Cm2 = wtiles(wp, "cmerge_w", D, D, D)
            CD = wp.tile([P, CT, D], F32, name="CD")
            Wc2 = wp.tile([HD, 2 * H, D], F32, name="Wc2")
            with tc.tile_pool(name="w2tmp", bufs=1) as wt:
                C2 = wtiles(wt, "cknn_w", D, D, D)
                for c in range(CT):
                    nc.vector.tensor_tensor(out=RR(CD[:, c, :]), in0=C2[:, c, :],
                                            in1=C1[:, c, :], op=OP.subtract)
                Cm1 = wtiles(wt, "cmerge_w", D, D, 0)
                combine(wt, Wc2, "cproj_w", Cm1, "cproj_b", "cmerge_b", 1)

            block(2, q2_d, nqc, v_loc, nvc,
                  (qw_t, 0), (kw_t, 0), (vw_t, 0),
                  C1, CD, cknn_b_t, Wc2, Cm2, 1,
                  idx2_sb, y2_loc, y2_full, q2_d, q3_d)

        # ================= FFN =================
        with tc.tile_pool(name="w3pool", bufs=1) as wp:
            fc1_w = wtiles(wp, "fc1_w", D, 4 * D)
            fc2_w = wtiles(wp, "fc2_w", 4 * D, D)
            hT = wp.tile([P, CT, N], F32, name="hT")
            for b in range(BS):
                for nt in range(NT):
                    def load_q3(b=b, nt=nt):
                        t = work.tile([P, D], F32, name="q3_ld", tag="x_ld")
                        nc.sync.dma_start(
                            out=t[:],
                            in_=q3_d.ap()[
                                (b * NT + nt) * P:(b * NT + nt + 1) * P, :])
                        return t[:]
                    ln_transpose(load_q3, n2c, hT, nt)
                for ch in range(2):
                    G = wp.tile([P, FT, 512], F32, name="G", tag="G")
                    for ot in range(FT):
                        pm = psB.tile([P, 512], F32, name="fc1_ps", tag="mmB")
                        mm_acc(pm[:],
                               [fc1_w[:, ct, ot * P:(ot + 1) * P]
                                for ct in range(CT)],
                               [hT[:, ct, ch * 512:(ch + 1) * 512]
                                for ct in range(CT)])
                        nc.scalar.activation(out=RR(G[:, ot, :]), in_=pm[:],
                                             func=AF.Gelu,
                                             bias=fc1b[:, ot:ot + 1], scale=1.0)
                    for snt in range(4):
                        nt = ch * 4 + snt
                        pm2 = psB.tile([P, D], F32, name="fc2_ps", tag="mmB")
                        for ot in range(FT):
                            nc.tensor.matmul(
                                pm2[:],
                                lhsT=RR(G[:, ot, snt * P:(snt + 1) * P]),
                                rhs=RR(fc2_w[:, ot, :]),
                                start=(ot == 0), stop=(ot == FT - 1))
                        qin = work.tile([P, D], F32, name="ffn_qin", tag="x_ld")
                        nc.sync.dma_start(
                            out=qin[:],
                            in_=q3_d.ap()[
                                (b * NT + nt) * P:(b * NT + nt + 1) * P, :])
                        t = work.tile([P, D], F32, name="ffn_t", tag="tmpA")
                        nc.vector.tensor_tensor(out=t[:], in0=pm2[:],
                                                in1=fc2_b_t[:], op=OP.add)
                        nc.vector.tensor_tensor(out=t[:], in0=t[:], in1=qin[:],
                                                op=OP.add)
                        nc.sync.dma_start(
                            out=out.ap()[
                                (b * NT + nt) * P:(b * NT + nt + 1) * P, :],
                            in_=t[:])


# ---------------- host wrapper ----------------
_NC_CACHE = None


def _get_nc():
    global _NC_CACHE
    if _NC_CACHE is None:
        _NC_CACHE = build_module()
    return _NC_CACHE


def _arrange_idx(idx_global, core):
    """(B*K*N,) global -> per-core [128, 128] int32; column g =
    b_local*64 + k*8 + nt holds the 128 indices for rows nt*128..+128."""
    a = np.asarray(idx_global).reshape(B, K, N)[core * BS:(core + 1) * BS]
    a = a.reshape(BS, K, NT, P).transpose(3, 0, 1, 2).reshape(P, P)
    return np.ascontiguousarray(a.astype(np.int32))


def kernel(q, v, knn_index, cross_knn_index, params):
    nc = _get_nc()
    q = np.asarray(q, dtype=np.float32)
    v = np.asarray(v, dtype=np.float32)

    wmap = {k: np.ascontiguousarray(np.asarray(p, dtype=np.float32))
            for k, p in params.items()}

    in_maps = []
    for c in range(NCORES):
        m = dict(wmap)
        m["q_loc"] = np.ascontiguousarray(q[c * BS:(c + 1) * BS].reshape(NL, D))
        m["v_loc"] = np.ascontiguousarray(v[c * BS:(c + 1) * BS].reshape(NL, D))
        m["idx1"] = _arrange_idx(knn_index, c)
        m["idx2"] = _arrange_idx(cross_knn_index, c)
        in_maps.append(m)

    res = bass_utils.run_bass_kernel_spmd(nc, in_maps,
                                          core_ids=list(range(NCORES)))
    out = np.stack([res.results[c]["out"].reshape(BS, N, D)
                    for c in range(NCORES)])
    return np.ascontiguousarray(out.reshape(B, N, D))
